# revision 7
# baseline (speedup 1.0000x reference)
"""Trainium2 Bass kernel for nn_DecoderRNNFeed (attention-GRU decoder).

Strategy: pure data-parallel over batch (8 rows per NeuronCore, zero
cross-core communication — collectives have a multi-microsecond floor per
call which is unusable inside a 128-step recurrence).

Per-core dataflow (everything in "transposed" layout, batch on the free dim):
  - encoder_outputs for this core's 8 batch rows live in SBUF (fp8e4, 8 MB).
  - The embedding contribution to the attention logits and to the combine
    layer is precomputed on-device (gather + two 1024x1024x1024 GEMMs) into
    DRAM scratch, then streamed in per step as an [8, 1024] tile whose
    transpose is injected into PSUM with an identity-matmul trick.
  - Per timestep: logitsT = attn_Wh-stationary MMs + inject; softmax without
    max-subtraction (logits are small by construction); ctx via
    enc-stationary matvec MMs; combine + GRU gates with weight-stationary
    MMs; gate math on DVE/ACT in transposed layout (no transposes needed).
  - Outputs: unnormalized exp(logits) per step (fp32) + final hidden state;
    the host normalizes the softmax and reassembles layouts (zero-FLOP
    transforms beyond the softmax division).

All matmul FLOPs of the reference (embedding projections, attention matmul,
context matvec, combine, GRU cell) run on the NeuronCores.
"""

import numpy as np
import ml_dtypes

import concourse.bass as bass
import concourse.tile as tile
import concourse.mybir as mybir
from concourse.bass_utils import run_bass_kernel_spmd

F32 = mybir.dt.float32
F16 = mybir.dt.float16
F8 = mybir.dt.float8e4
I32 = mybir.dt.int32

N_CORES = 8
B = 64
BL = B // N_CORES          # batch rows per core = 8
H = 1024                   # hidden size (= L = E)
L = 1024
E = 1024
V = 32000
HT = H // 128              # 8 tiles

CFG = dict(
    T=128,                 # timesteps
    U=2,                   # steps per loop body
    enc_dt=F16,            # encoder_outputs storage dtype in SBUF
    w_dt=F16,              # W_ih / W_hh stationary dtype (streamed from HBM)
    at_dt=F16,             # attn_Wh / comb_Wctx stationary dtype
)

MAX_WAITS = 1


def _split_multiwaits(nc):
    """This container's walrus rejects >1 sync-wait per instruction; Tile
    attaches one wait per processor to kernel-tail / back-edge Drains. Move
    excess waits onto carrier NoOps inserted just before, same engine."""
    for fn in nc.m.functions:
        for blk in fn.blocks:
            insts = list(blk.instructions)
            new = []
            changed = False
            for inst in insts:
                si = inst.sync_info
                if si is not None and si.on_wait and len(si.on_wait) > MAX_WAITS:
                    waits = list(si.on_wait)
                    for w in waits[MAX_WAITS:]:
                        nop = mybir.InstNoOp(
                            name=nc.get_next_instruction_name(),
                            engine=inst.engine,
                            sync_info=mybir.SyncInfo(on_wait=[w], on_update=[]),
                            bass_nofuse=True,
                        )
                        new.append(nop)
                    inst.sync_info = mybir.SyncInfo(
                        on_wait=waits[:MAX_WAITS], on_update=list(si.on_update)
                    )
                    changed = True
                new.append(inst)
            if changed:
                blk.instructions = new


# ---------------------------------------------------------------- host prep

def _pack_lhsT(W, n_kt, n_mt, dt):
    """W is [M_total, K_total] (row-major out-dim first, as in W @ x.T use).
    Returns [128, n_kt*n_mt*128] where tile (kt, mt) sits at
    cols (kt*n_mt + mt)*128 and holds W[mt*128+m, kt*128+k] at [k, m]."""
    M, K = W.shape
    assert M == n_mt * 128 and K == n_kt * 128
    W4 = W.reshape(n_mt, 128, n_kt, 128)            # [mt, m, kt, k]
    return np.ascontiguousarray(
        W4.transpose(3, 2, 0, 1).reshape(128, n_kt * n_mt * 128)
    ).astype(dt)


def _pack_lhsT_mtmajor(W, n_kt, n_mt, dt):
    """Stream packing: tile (kt, mt) at cols (mt*n_kt + kt)*128 so a chunk of
    consecutive mt values is one contiguous slice."""
    M, K = W.shape
    assert M == n_mt * 128 and K == n_kt * 128
    W4 = W.reshape(n_mt, 128, n_kt, 128)            # [mt, m, kt, k]
    return np.ascontiguousarray(
        W4.transpose(3, 0, 2, 1).reshape(128, n_kt * n_mt * 128)
    ).astype(dt)


def _pack_rhs_T(W, dt):
    """W is [N_total, K=1024] ; returns [128, 8*N_total] with
    out[k_lo, kt*N_total + n] = W[n, kt*128+k_lo] (K on partitions)."""
    N, K = W.shape
    assert K == 1024
    W3 = W.reshape(N, 8, 128)                        # [n, kt, k_lo]
    return np.ascontiguousarray(
        W3.transpose(2, 1, 0).reshape(128, 8 * N)
    ).astype(dt)


def prep_core_inputs(core, tgt_input, hidden, encoder_outputs, emb16,
                     attn_pk, attnWeT, attnb16, comb_pk, combWeT, combb16,
                     wih_pk, whh_pk, consts, cfg):
    b0 = core * BL
    T = cfg["T"]
    # embedding gather indices, row r = t*BL + b  (t-major)
    tgt = np.asarray(tgt_input)[b0:b0 + BL]          # [8, T_in]
    if tgt.shape[1] < T:                             # timing variants only
        tgt = np.tile(tgt, (1, (T + tgt.shape[1] - 1) // tgt.shape[1]))
    tgt = tgt[:, :T]
    idx = np.ascontiguousarray(tgt.T).reshape(T * BL, 1).astype(np.int32)
    # initial hidden, transposed-packed [h_lo, ht*8+b]
    h0 = np.asarray(hidden)[0, b0:b0 + BL, :]        # [8, 1024]
    hT0 = np.ascontiguousarray(
        h0.reshape(BL, HT, 128).transpose(2, 1, 0).reshape(128, HT * BL)
    ).astype(np.float32)
    # encoder outputs fp8-packed: tile (b, lt, ht) at ((b*8+lt)*8+ht)*128
    enc = np.asarray(encoder_outputs)[b0:b0 + BL]    # [8, 1024, 1024]
    enc5 = enc.reshape(BL, 8, 128, 8, 128)           # [b, lt, l_lo, ht, h_lo]
    enc_pk = np.ascontiguousarray(
        enc5.transpose(2, 0, 1, 3, 4).reshape(128, BL * 8 * 8 * 128)
    ).astype(mybir.dt.np(cfg["enc_dt"]))
    return {
        "embidx": idx,
        "hT0": hT0,
        "enc_pk": enc_pk,
        "emb16": emb16,
        "attnh_pk": attn_pk,
        "attnWeT_pk": attnWeT,
        "attnb16": attnb16,
        "combc_pk": comb_pk,
        "combWeT_pk": combWeT,
        "combb16": combb16,
        "wih_pk": wih_pk,
        "whh_pk": whh_pk,
        **consts,
    }


def prep_inputs(inputs, cfg):
    """Returns in_maps (list of 8 per-core dicts)."""
    emb16 = np.asarray(inputs["emb_table"]).astype(np.float16)
    attn_W = np.asarray(inputs["attn_W"], dtype=np.float32)   # [L, E+H]
    comb_W = np.asarray(inputs["comb_W"], dtype=np.float32)   # [H, E+H]
    at_np = mybir.dt.np(cfg["at_dt"])
    w_np = mybir.dt.np(cfg["w_dt"])
    attn_pk = _pack_lhsT(attn_W[:, E:], HT, 8, at_np)         # attn_Wh^T tiles
    comb_pk = _pack_lhsT(comb_W[:, E:], HT, 8, at_np)         # comb_Wctx^T
    attnWeT = _pack_rhs_T(attn_W[:, :E], np.float16)          # [128, 8*1024]
    combWeT = _pack_rhs_T(comb_W[:, :E], np.float16)
    attnb16 = np.asarray(inputs["attn_b"]).reshape(1, L).astype(np.float16)
    combb16 = np.asarray(inputs["comb_b"]).reshape(1, H).astype(np.float16)
    wih_pk = _pack_lhsT_mtmajor(np.asarray(inputs["W_ih"], np.float32), HT, 24, w_np)
    whh_pk = _pack_lhsT_mtmajor(np.asarray(inputs["W_hh"], np.float32), HT, 24, w_np)
    consts = {
        "i8c": np.eye(8, dtype=np.float16),
        "i128c": np.eye(128, dtype=np.float16),
        "ones_row16": np.ones((1, 128), dtype=np.float16),
        "ones_col32": np.ones((128, 1), dtype=np.float32),
        "ones_row32": np.ones((1, 128), dtype=np.float32),
        "bih16": np.asarray(inputs["b_ih"]).reshape(1, 3 * H).astype(np.float16),
        "bhh16": np.asarray(inputs["b_hh"]).reshape(1, 3 * H).astype(np.float16),
    }
    return [
        prep_core_inputs(c, inputs["tgt_input"], inputs["hidden"],
                         inputs["encoder_outputs"], emb16, attn_pk, attnWeT,
                         attnb16, comb_pk, combWeT, combb16, wih_pk, whh_pk,
                         consts, cfg)
        for c in range(N_CORES)
    ]


# ------------------------------------------------------------- bass program

def build(nc, cfg):
    T, U = cfg["T"], cfg["U"]
    enc_dt, w_dt, at_dt = cfg["enc_dt"], cfg["w_dt"], cfg["at_dt"]
    assert T % U == 0

    # DRAM parameters
    embidx_d = nc.dram_tensor("embidx", [T * BL, 1], I32, kind="ExternalInput")
    hT0_d = nc.dram_tensor("hT0", [128, HT * BL], F32, kind="ExternalInput")
    enc_d = nc.dram_tensor("enc_pk", [128, BL * 8 * 8 * 128], enc_dt, kind="ExternalInput")
    emb_d = nc.dram_tensor("emb16", [V, E], F16, kind="ExternalInput")
    attnh_d = nc.dram_tensor("attnh_pk", [128, HT * 8 * 128], at_dt, kind="ExternalInput")
    attnWeT_d = nc.dram_tensor("attnWeT_pk", [128, 8 * L], F16, kind="ExternalInput")
    attnb_d = nc.dram_tensor("attnb16", [1, L], F16, kind="ExternalInput")
    combc_d = nc.dram_tensor("combc_pk", [128, HT * 8 * 128], at_dt, kind="ExternalInput")
    combWeT_d = nc.dram_tensor("combWeT_pk", [128, 8 * H], F16, kind="ExternalInput")
    combb_d = nc.dram_tensor("combb16", [1, H], F16, kind="ExternalInput")
    wih_d = nc.dram_tensor("wih_pk", [128, HT * 24 * 128], w_dt, kind="ExternalInput")
    whh_d = nc.dram_tensor("whh_pk", [128, HT * 24 * 128], w_dt, kind="ExternalInput")
    i8_d = nc.dram_tensor("i8c", [8, 8], F16, kind="ExternalInput")
    i128_d = nc.dram_tensor("i128c", [128, 128], F16, kind="ExternalInput")
    onesr16_d = nc.dram_tensor("ones_row16", [1, 128], F16, kind="ExternalInput")
    onesc32_d = nc.dram_tensor("ones_col32", [128, 1], F32, kind="ExternalInput")
    onesr32_d = nc.dram_tensor("ones_row32", [1, 128], F32, kind="ExternalInput")
    bih_d = nc.dram_tensor("bih16", [1, 3 * H], F16, kind="ExternalInput")
    bhh_d = nc.dram_tensor("bhh16", [1, 3 * H], F16, kind="ExternalInput")

    expT_out = nc.dram_tensor("expT_out", [T * 128, 8 * BL], F32, kind="ExternalOutput")
    hT_out = nc.dram_tensor("hT_out", [128, HT * BL], F32, kind="ExternalOutput")

    elog_d = nc.dram_tensor("elog_d", [T * BL, L], F16)
    ecomb_d = nc.dram_tensor("ecomb_d", [T * BL, H], F16)

    from contextlib import ExitStack
    with tile.TileContext(nc) as tc, ExitStack() as est:
        # ---- persistent constants & state
        consts = est.enter_context(tc.tile_pool(name="consts", bufs=1))
        i8_t = consts.tile([8, 8], F16)
        nc.sync.dma_start(i8_t[:], i8_d[:])
        onesc32_t = consts.tile([128, 1], F32)
        nc.sync.dma_start(onesc32_t[:], onesc32_d[:])
        onesr32_t = consts.tile([1, 128], F32)
        nc.sync.dma_start(onesr32_t[:], onesr32_d[:])
        hT32 = consts.tile([128, HT * BL], F32)
        nc.sync.dma_start(hT32[:], hT0_d[:])
        hT16 = consts.tile([128, HT * BL], F16)
        nc.vector.tensor_copy(hT16[:], hT32[:])

        # ---- persistent weights
        wpool = est.enter_context(tc.tile_pool(name="weights", bufs=1))
        attnh_sb = wpool.tile([128, HT * 8 * 128], at_dt)
        nc.sync.dma_start(attnh_sb[:], attnh_d[:])
        combc_sb = wpool.tile([128, HT * 8 * 128], at_dt)
        nc.sync.dma_start(combc_sb[:], combc_d[:])
        # GRU biases, transposed-packed [128, 24*8] via host layout trick:
        # bias col for (mt, b) is bias[mt*128 + m_lo] broadcast over b. We
        # inject biases by adding them to the e_log/e_comb precompute instead
        # (b_ih + b_hh fold into the gates identically for every t, but they
        # enter *after* the nonlinear r gate for h_n... so fold b_ih into
        # e_comb is wrong; keep gate biases as explicit tiles instead).
        biasg_sb = wpool.tile([128, 24 * BL], F32)  # [m_lo, mt*8+b] = b_ih+b_hh
        bihT = wpool.tile([128, 24 * BL], F32)      # b_ih only (for i_n)
        bhhT = wpool.tile([128, 24 * BL], F32)      # b_hh only (for h_n)
        # build from [1, 3072] via broadcast matmul: ones_col32 @ bias_row —
        # simpler: host packs them; but to avoid more inputs, build with PE:
        # biasT[m_lo, (mt, b)] = bias[mt*128+m_lo]; do it with DMA access
        # patterns instead: DMA [1, 3072] -> [128, 24] (transpose-ish), then
        # DVE-broadcast along b. Simplest robust path: host-side extra inputs.
        # (bih_pk / bhh_pk are added in prep via consts dict update below.)

        with tc.tile_pool(name="prologue", bufs=1) as pro, \
             tc.tile_pool(name="chunks", bufs=2) as chp, \
             tc.tile_pool(name="pps", bufs=2, space="PSUM") as pps:
            i128_t = pro.tile([128, 128], F16)
            nc.sync.dma_start(i128_t[:], i128_d[:])
            onesr16_t = pro.tile([1, 128], F16)
            nc.sync.dma_start(onesr16_t[:], onesr16_d[:])
            attnb_t = pro.tile([1, L], F16)
            nc.sync.dma_start(attnb_t[:], attnb_d[:])
            combb_t = pro.tile([1, H], F16)
            nc.sync.dma_start(combb_t[:], combb_d[:])
            attnWeT_sb = pro.tile([128, 8 * L], F16)
            nc.sync.dma_start(attnWeT_sb[:], attnWeT_d[:])
            combWeT_sb = pro.tile([128, 8 * H], F16)
            nc.sync.dma_start(combWeT_sb[:], combWeT_d[:])
            embT_sb = pro.tile([128, 8 * T * BL], F16)  # [e_lo, et*TB + row]
            TB = T * BL

            # build gate-bias tiles [128, 24*8]: biasT[m_lo, mt*8+b] =
            # b(mt*128+m_lo), same for every b: matmul trick:
            # lhsT = bias_slice [1, 128] (K=1, M=128), rhs = ones_row16[1, 8]
            bih_t = pro.tile([1, 3 * H], F16)
            nc.sync.dma_start(bih_t[:], bih_d[:])
            bhh_t = pro.tile([1, 3 * H], F16)
            nc.sync.dma_start(bhh_t[:], bhh_d[:])
            bps = pps.tile([128, 24 * BL], F32, tag="bps")
            for mt in range(24):
                nc.tensor.matmul(
                    bps[:, mt * BL:(mt + 1) * BL],
                    bih_t[:1, mt * 128:(mt + 1) * 128],
                    onesr16_t[:1, :BL], start=True, stop=True)
            nc.vector.tensor_copy(bihT[:], bps[:])
            bps2 = pps.tile([128, 24 * BL], F32, tag="bps")
            for mt in range(24):
                nc.tensor.matmul(
                    bps2[:, mt * BL:(mt + 1) * BL],
                    bhh_t[:1, mt * 128:(mt + 1) * 128],
                    onesr16_t[:1, :BL], start=True, stop=True)
            nc.vector.tensor_copy(bhhT[:], bps2[:])
            nc.vector.tensor_tensor(
                biasg_sb[:], bihT[:], bhhT[:], op=mybir.AluOpType.add)

            # gather embeddings + transpose into embT_sb
            n_chunks = TB // 128
            for rt in range(n_chunks):
                idx_t = chp.tile([128, 1], I32, tag="idx")
                nc.sync.dma_start(idx_t[:], embidx_d[rt * 128:(rt + 1) * 128, :])
                embch = chp.tile([128, E], F16, tag="embch")
                nc.gpsimd.indirect_dma_start(
                    out=embch[:],
                    out_offset=None,
                    in_=emb_d[:, :],
                    in_offset=bass.IndirectOffsetOnAxis(ap=idx_t[:, :1], axis=0),
                )
                for et in range(8):
                    tp = pps.tile([128, 128], F16, tag="tp")
                    nc.tensor.transpose(
                        tp[:], embch[:, et * 128:(et + 1) * 128], i128_t[:])
                    nc.vector.tensor_copy(
                        embT_sb[:, et * TB + rt * 128: et * TB + (rt + 1) * 128],
                        tp[:])

            # P1/P2: e_log = emb @ attn_We.T + attn_b ; e_comb likewise
            pre_stores = []
            for (WeT_sb, b_t, out_d) in (
                (attnWeT_sb, attnb_t, elog_d),
                (combWeT_sb, combb_t, ecomb_d),
            ):
                for rt in range(n_chunks):
                    for nh in range(2):
                        pp = pps.tile([128, 512], F32, tag="pp")
                        nc.tensor.matmul(
                            pp[:], onesr16_t[:1, :128],
                            b_t[:1, nh * 512:(nh + 1) * 512],
                            start=True, stop=False)
                        for et in range(8):
                            nc.tensor.matmul(
                                pp[:],
                                embT_sb[:, et * TB + rt * 128: et * TB + (rt + 1) * 128],
                                WeT_sb[:, et * L + nh * 512: et * L + nh * 512 + 512],
                                start=False, stop=(et == 7))
                        st = chp.tile([128, 512], F16, tag="pst")
                        nc.vector.tensor_copy(st[:], pp[:])
                        pre_stores.append(nc.sync.dma_start(
                            out_d[rt * 128:(rt + 1) * 128, nh * 512:(nh + 1) * 512],
                            st[:]))

        # barrier: make sure P1/P2 DRAM writes land before loop reads them
        # (Tile's dep tracking is tile-based; DRAM round-trip needs this).
        tc.strict_bb_all_engine_barrier()

        # encoder_outputs into SBUF (after prologue pools released)
        encp = est.enter_context(tc.tile_pool(name="encp", bufs=1))
        enc_sb = encp.tile([128, BL * 8 * 8 * 128], enc_dt)
        nc.sync.dma_start(enc_sb[:], enc_d[:])

        # ---- main recurrence
        lp = est.enter_context(tc.tile_pool(name="lp", bufs=1))
        wstr = est.enter_context(tc.tile_pool(name="wstr", bufs=4))
        ps = est.enter_context(tc.tile_pool(name="ps", bufs=1, space="PSUM"))

        stage_el = [lp.tile([BL, L], F16, name=f"stel{u}") for u in range(U)]
        stage_ec = [lp.tile([BL, H], F16, name=f"stec{u}") for u in range(U)]

        with tc.For_i(0, T, U) as iv:
            for u in range(U):
                ld1 = nc.sync.dma_start(
                    stage_el[u][:], elog_d[bass.ds(iv * BL + u * BL, BL), :])
                ld2 = nc.sync.dma_start(
                    stage_ec[u][:], ecomb_d[bass.ds(iv * BL + u * BL, BL), :])
                for s in pre_stores:
                    tile.add_dep_helper(ld1.ins, s.ins, sync=True,
                                        reason="DRAM RAW: elog/ecomb")
                    tile.add_dep_helper(ld2.ins, s.ins, sync=True,
                                        reason="DRAM RAW: elog/ecomb")
            for u in range(U):
                # logitsT [l_lo, (lt, b)]
                lg = ps.tile([128, 8 * BL], F32, tag="lg")
                for lt in range(8):
                    sl = slice(lt * BL, (lt + 1) * BL)
                    nc.tensor.matmul(
                        lg[:, sl], stage_el[u][:, lt * 128:(lt + 1) * 128],
                        i8_t[:], start=True, stop=False)
                    for kt in range(HT):
                        nc.tensor.matmul(
                            lg[:, sl],
                            attnh_sb[:, (kt * 8 + lt) * 128:(kt * 8 + lt + 1) * 128],
                            hT16[:, kt * BL:(kt + 1) * BL],
                            start=False, stop=(kt == HT - 1))
                expt = lp.tile([128, 8 * BL], F32, tag="expt", bufs=2)
                nc.scalar.activation(expt[:], lg[:], mybir.ActivationFunctionType.Exp)
                nc.sync.dma_start(
                    expT_out[bass.ds(iv * 128 + u * 128, 128), :], expt[:])
                # softmax sums per b: ones.T @ expt -> [1, (lt, b)]
                sm = ps.tile([1, 8 * BL], F32, tag="sm")
                nc.tensor.matmul(sm[:], onesc32_t[:], expt[:], start=True, stop=True)
                rsum = lp.tile([1, BL], F32, tag="rsum")
                nc.vector.reduce_sum(
                    rsum[:1, :],
                    sm[:1, :].rearrange("p (lt b) -> p b lt", b=BL),
                    axis=mybir.AxisListType.X)
                recip = lp.tile([1, BL], F32, tag="recip")
                nc.vector.reciprocal(recip[:1, :], rsum[:1, :])
                rbc = ps.tile([128, BL], F32, tag="rbc")
                nc.tensor.matmul(rbc[:], onesr32_t[:1, :], recip[:1, :],
                                 start=True, stop=True)
                awT = lp.tile([128, 8 * BL], F16, tag="awT", bufs=2)
                nc.vector.tensor_tensor(
                    awT[:].rearrange("p (lt b) -> p lt b", b=BL),
                    expt[:].rearrange("p (lt b) -> p lt b", b=BL),
                    rbc[:, None, :BL].broadcast_to([128, 8, BL]),
                    op=mybir.AluOpType.mult)
                # ctx: ctxT [h_lo, (ht, b)]
                cx = ps.tile([128, HT * BL], F32, tag="cx")
                for b in range(BL):
                    for ht in range(HT):
                        col = ht * BL + b
                        for lt in range(8):
                            enc_off = ((b * 8 + lt) * 8 + ht) * 128
                            nc.tensor.matmul(
                                cx[:, col:col + 1],
                                enc_sb[:, enc_off:enc_off + 128],
                                awT[:, lt * BL + b: lt * BL + b + 1],
                                start=(lt == 0), stop=(lt == 7))
                ctx16 = lp.tile([128, HT * BL], F16, tag="ctx16", bufs=2)
                nc.scalar.activation(
                    ctx16[:], cx[:], mybir.ActivationFunctionType.Copy)
                # combine: xT = relu(e_comb^T + Wctx @ ctx)
                xp = ps.tile([128, HT * BL], F32, tag="xp")
                for ht in range(HT):
                    sl = slice(ht * BL, (ht + 1) * BL)
                    nc.tensor.matmul(
                        xp[:, sl], stage_ec[u][:, ht * 128:(ht + 1) * 128],
                        i8_t[:], start=True, stop=False)
                    for kt in range(HT):
                        nc.tensor.matmul(
                            xp[:, sl],
                            combc_sb[:, (kt * 8 + ht) * 128:(kt * 8 + ht + 1) * 128],
                            ctx16[:, kt * BL:(kt + 1) * BL],
                            start=False, stop=(kt == HT - 1))
                xT16 = lp.tile([128, HT * BL], F16, tag="xT16", bufs=2)
                nc.scalar.activation(
                    xT16[:], xp[:], mybir.ActivationFunctionType.Relu)
                # GRU gates (transposed): rz [128, 16*8], i_n/h_n [128, 8*8]
                # W_ih / W_hh are streamed from HBM in mt-major chunks of
                # 3 output-tiles (6KB/partition); each mt accumulation group
                # (8 ih MMs [+ 8 hh MMs for r/z]) runs consecutively.
                rzp = ps.tile([128, 16 * BL], F32, tag="rzp")
                inp = ps.tile([128, 8 * BL], F32, tag="inp")
                hnp = ps.tile([128, 8 * BL], F32, tag="hnp")
                CH = 3 * HT * 128  # cols per 3-mt chunk
                for c in range(8):
                    wic = wstr.tile([128, CH], w_dt, name=f"wic", tag="wstr", bufs=4)
                    nc.sync.dma_start(wic[:], wih_d[:, c * CH:(c + 1) * CH])
                    whc = wstr.tile([128, CH], w_dt, name=f"whc", tag="wstr", bufs=4)
                    nc.sync.dma_start(whc[:], whh_d[:, c * CH:(c + 1) * CH])
                    for j in range(3):
                        mt = c * 3 + j
                        if mt < 16:
                            out_ih = out_hh = rzp[:, mt * BL:(mt + 1) * BL]
                            st_ih, sp_ih = True, False
                            st_hh, sp_hh = False, True
                        else:
                            out_ih = inp[:, (mt - 16) * BL:(mt - 15) * BL]
                            out_hh = hnp[:, (mt - 16) * BL:(mt - 15) * BL]
                            st_ih = sp_ih = st_hh = sp_hh = True
                        for kt in range(HT):
                            nc.tensor.matmul(
                                out_ih,
                                wic[:, (j * HT + kt) * 128:(j * HT + kt + 1) * 128],
                                xT16[:, kt * BL:(kt + 1) * BL],
                                start=(st_ih and kt == 0),
                                stop=(sp_ih and kt == HT - 1))
                        for kt in range(HT):
                            nc.tensor.matmul(
                                out_hh,
                                whc[:, (j * HT + kt) * 128:(j * HT + kt + 1) * 128],
                                hT16[:, kt * BL:(kt + 1) * BL],
                                start=(st_hh and kt == 0),
                                stop=(sp_hh and kt == HT - 1))
                # gate math; sigmoid(x) = 0.5*tanh(0.5x)+0.5 (stay in one
                # ACT table set: exp/tanh/relu/copy)
                rzpb = lp.tile([128, 16 * BL], F32, tag="rzpb")
                nc.vector.tensor_tensor(
                    rzpb[:], rzp[:], biasg_sb[:, :16 * BL],
                    op=mybir.AluOpType.add)
                th = lp.tile([128, 16 * BL], F32, tag="th")
                nc.scalar.activation(
                    th[:], rzpb[:], mybir.ActivationFunctionType.Tanh, scale=0.5)
                rz = lp.tile([128, 16 * BL], F32, tag="rz")
                nc.vector.tensor_scalar(
                    rz[:], th[:], 0.5, 0.5,
                    op0=mybir.AluOpType.mult, op1=mybir.AluOpType.add)
                # n = tanh(i_n + b_ih_n + r*(h_n + b_hh_n))
                hnb = lp.tile([128, 8 * BL], F32, tag="hnb")
                nc.vector.tensor_tensor(
                    hnb[:], hnp[:], bhhT[:, 16 * BL:],
                    op=mybir.AluOpType.add)
                rhn = lp.tile([128, 8 * BL], F32, tag="rhn")
                nc.vector.tensor_tensor(
                    rhn[:], rz[:, :8 * BL], hnb[:], op=mybir.AluOpType.mult)
                inb = lp.tile([128, 8 * BL], F32, tag="inb")
                nc.vector.tensor_tensor(
                    inb[:], inp[:], bihT[:, 16 * BL:], op=mybir.AluOpType.add)
                npre = lp.tile([128, 8 * BL], F32, tag="npre")
                nc.vector.tensor_tensor(
                    npre[:], inb[:], rhn[:], op=mybir.AluOpType.add)
                nT = lp.tile([128, 8 * BL], F32, tag="nT")
                nc.scalar.activation(
                    nT[:], npre[:], mybir.ActivationFunctionType.Tanh)
                # h' = n + z*(h - n)
                d_t = lp.tile([128, 8 * BL], F32, tag="d_t")
                nc.vector.tensor_tensor(
                    d_t[:], hT32[:], nT[:], op=mybir.AluOpType.subtract)
                zd = lp.tile([128, 8 * BL], F32, tag="zd")
                nc.vector.tensor_tensor(
                    zd[:], rz[:, 8 * BL:], d_t[:], op=mybir.AluOpType.mult)
                nc.vector.tensor_tensor(
                    hT32[:], nT[:], zd[:], op=mybir.AluOpType.add)
                nc.vector.tensor_copy(hT16[:], hT32[:])

        nc.sync.dma_start(hT_out[:, :], hT32[:])

    return nc


# ------------------------------------------------------------------ driver

def _postprocess(results, cfg):
    T = cfg["T"]
    aws = np.empty((B, T, L), dtype=np.float32)
    hfin = np.empty((B, H), dtype=np.float32)
    for c, res in enumerate(results):
        b0 = c * BL
        ex = res["expT_out"].reshape(T, 128, 8, BL)      # [t, l_lo, lt, b]
        sums = ex.sum(axis=(1, 2))                       # [t, b]
        aw = ex.transpose(3, 0, 2, 1).reshape(BL, T, L)  # [b, t, lt*128+l_lo]
        aws[b0:b0 + BL] = aw / sums.T[:, :, None]
        hT = res["hT_out"].reshape(128, HT, BL)          # [h_lo, ht, b]
        hfin[b0:b0 + BL] = hT.transpose(2, 1, 0).reshape(BL, H)
    return aws, hfin


_BUILD_CACHE = {}


def run(inputs, cfg=None, **spmd_kwargs):
    cfg = cfg or CFG
    key = tuple(sorted((k, str(v)) for k, v in cfg.items()))
    if key not in _BUILD_CACHE:
        nc = bass.Bass("TRN2", target_bir_lowering=False, debug=False)
        build(nc, cfg)
        _split_multiwaits(nc)
        _BUILD_CACHE[key] = nc
    nc = _BUILD_CACHE[key]
    in_maps = prep_inputs(inputs, cfg)
    res = run_bass_kernel_spmd(nc, in_maps, core_ids=list(range(N_CORES)),
                               **spmd_kwargs)
    aws, hfin = _postprocess(res.results, cfg)
    return aws, hfin, res


def kernel(tgt_input, hidden, encoder_outputs, batch_size, emb_table,
           attn_W, attn_b, comb_W, comb_b, W_ih, b_ih, W_hh, b_hh):
    inputs = dict(tgt_input=tgt_input, hidden=hidden,
                  encoder_outputs=encoder_outputs, batch_size=batch_size,
                  emb_table=emb_table, attn_W=attn_W, attn_b=attn_b,
                  comb_W=comb_W, comb_b=comb_b, W_ih=W_ih, b_ih=b_ih,
                  W_hh=W_hh, b_hh=b_hh)
    aws, hfin, _ = run(inputs, CFG)
    return aws, hfin[None], aws
